# revision 55
# baseline (speedup 1.0000x reference)
"""Deformable conv (3x3, pad=1, B=8, Cin=Cout=256, H=W=64) on 8 TRN2 NeuronCores.

Strategy (data-parallel over batch, one image per core):
  1. Prologue per core: build a zero-padded *superpixel* image IZ in HBM via PE
     transpose: IZ[R, X] = [pix(R-1, X-1), pix(R, X-1)] (vertical pixel pair,
     512 ch contiguous, bf16), R in 0..64, X in 0..65, with zero rows/cols on
     the borders.  One 2KB gather descriptor starting at superpixel (R, X)
     then fetches all 4 bilinear corners [A0 B0 A1 B1] of a tap sample.
     Also: transpose the (k,c)-ordered weight matrix into lhsT layout; compute
     per-token superpixel indices + 4 corner weights on DVE from the offsets.
  2. Main loop over (token-chunk, tap): ONE SWDGE dma_gather fetches the 4
     corners per output token (token-major: token on partition, 4x256 channels
     on free dim).  DVE multiplies the corners by per-token weights (free-dim
     step-0 broadcast APs with dup-pairs for the 2x_1p perf mode) and does the
     first pair-sum; GPSIMD does the second pair-sum.  PE block-transposes the
     blended tile to channel-major [c, token] in PSUM (ACT copies it back to
     SBUF), keeping the SDMA engines free for the main gather.  TensorE
     accumulates W_k^T @ cols into PSUM over all 9 taps x 2 c-halves.
  3. PSUM -> SBUF -> HBM fp32 output.

Boundary handling is entirely via the zero padding: the gather index is
(clip(y0,-1,63)+1)*66 + clip(x0,-1,63)+1 and out-of-image corners get weight 0
through the validity masks (exactly the reference's `valid` mask).
"""

import os

import numpy as np

import concourse.bacc as bacc
import concourse.bass as bass
import concourse.mybir as mybir
from concourse.bass import AP, ts
from concourse.bass_utils import run_bass_kernel_spmd
from concourse.masks import make_identity
from concourse.tile import TileContext

FP32 = mybir.dt.float32
BF16 = mybir.dt.bfloat16
I16 = mybir.dt.int16
I32 = mybir.dt.int32

B = 8
C = 256
H = W = 64
HW = H * W           # 4096 pixels / tokens per tap
K = 9                # 3x3 taps
COUT = 256
NCH = 1024           # tokens per chunk
NCHUNKS = HW // NCH  # 4
A = mybir.AluOpType

# superpixel table geometry: 65 store-rows (R=0..64), 66 store-cols (X=0..65),
# 2 pixel slots of 256 ch each.  flat [65*66*2, 256] bf16.
NROW = 65
NCOL = 66
SP_ELEMS = 2 * C                 # 512 elements per superpixel
IZ_ROWS = NROW * NCOL * 2        # 8580 rows of 256
ROW_STRIDE = NCOL * SP_ELEMS     # 33792 elems per store-row


def build_nc() -> bass.Bass:
    nc = bacc.Bacc(target_bir_lowering=False)

    x_in = nc.dram_tensor("x", [C, HW], FP32, kind="ExternalInput")
    off_in = nc.dram_tensor("off", [2 * K, HW], FP32, kind="ExternalInput")
    # host pre-permutes weight to [cout, (k, c)] layout
    w_in = nc.dram_tensor("w", [COUT, K * C], FP32, kind="ExternalInput")
    out_d = nc.dram_tensor("out", [COUT, HW], FP32, kind="ExternalOutput")

    # zero-padded superpixel image (see module docstring)
    iz_d = nc.dram_tensor("iz", [IZ_ROWS, C], BF16, kind="Internal")
    # token-major -> wrapped-16 index staging
    ilin_d = nc.dram_tensor("ilin", [128, K * HW // 128], I16, kind="Internal")

    with TileContext(nc) as tc:
        with tc.tile_pool(name="const", bufs=1) as cp:
            ident = cp.tile([128, 128], BF16)
            make_identity(nc, ident)
            identf = cp.tile([128, 128], FP32)
            make_identity(nc, identf)
            # lhsT tiles: block kc2 = k*2 + chalf holds rows c (128) x cout (256)
            wT = cp.tile([128, 2 * K, 256], BF16)
            # corner weights per token: (k, j, xpos, yrow, dup); bf16 dup
            # pairs give the blend's broadcast operand a stride-1 innermost
            # dim (enables the DVE 2x_1p perf mode)
            wpack = cp.tile([128, K, HW // 128, 2, 2, 2], BF16)
            # wrapped-16 gather indices per tap
            idx_w = cp.tile([128, K, HW // 16], I16)

            with (
                tc.tile_pool(name="prep", bufs=1) as pp,
                tc.tile_pool(name="ppsum", bufs=4, space="PSUM") as pps,
            ):
                # ---- x -> channels-last bf16 superpixel image in HBM ----
                x_bf = pp.tile([128, 2, HW], BF16)
                nc.gpsimd.dma_start(
                    out=x_bf[:], in_=x_in[:, :].rearrange("(h p) c -> p h c", p=128)
                )
                # zero borders first (rows -1/64 and cols -1/64 of the pad)
                zpad = pp.tile([128, 2 * C], BF16)
                nc.vector.memset(zpad[:], 0.0)
                # store-row 0, slot 0 (pixel row -1)
                nc.sync.dma_start(
                    out=AP(iz_d, 0, [[SP_ELEMS, NCOL], [1, C]]),
                    in_=zpad[0:NCOL, 0:C],
                )
                # store-row 64, slot 1 (pixel row 64)
                nc.sync.dma_start(
                    out=AP(iz_d, 64 * ROW_STRIDE + C, [[SP_ELEMS, NCOL], [1, C]]),
                    in_=zpad[0:NCOL, 0:C],
                )
                # store-col 0 (pixel col -1), both slots
                nc.sync.dma_start(
                    out=AP(iz_d, 0, [[ROW_STRIDE, NROW], [1, SP_ELEMS]]),
                    in_=zpad[0:NROW, :],
                )
                # store-col 65 (pixel col 64), both slots
                nc.sync.dma_start(
                    out=AP(iz_d, 65 * SP_ELEMS, [[ROW_STRIDE, NROW], [1, SP_ELEMS]]),
                    in_=zpad[0:NROW, :],
                )
                # ---- offsets -> indices + corner weights ----
                # (before the image transposes: the off->idx chain is the
                # critical path to the first gather; its 8 small PE transposes
                # go ahead of the 64 x-transposes, and the xcl PSUM->SBUF
                # copies run on ACT so DVE is free for this chain)
                # natural load (18 contiguous 16KB descriptors), then PE
                # transpose to token-major [128, j, r]
                off_nat = pp.tile([128, HW], FP32)
                nc.sync.dma_start(out=off_nat[0 : 2 * K, :], in_=off_in[:, :])
                off_t2 = pp.tile([128, HW // 128, 2 * K], FP32)
                for s4 in range(HW // 128 // 4):
                    ptf = pps.tile([128, 4, 2 * K], FP32, tag="ptf")
                    for i in range(4):
                        nc.tensor.transpose(
                            ptf[:, i, :],
                            off_nat[0 : 2 * K, ts(4 * s4 + i, 128)],
                            identf[0 : 2 * K, 0 : 2 * K],
                        )
                    nc.vector.tensor_copy(off_t2[:, 4 * s4 : 4 * s4 + 4, :], ptf[:])
                off_r = off_t2[:, :, :].rearrange("p j (k s) -> p s k j", s=2)
                oy = off_r[:, 0]  # [128, 9, 32] (k, j) strides (2, 18)
                ox = off_r[:, 1]

                NJ = HW // 128  # 32
                shp = [128, K, NJ]

                def f32(tag):
                    return pp.tile(shp, FP32, tag=tag, name=tag)

                # iotas
                it_j = pp.tile([128, NJ], I32)
                nc.gpsimd.iota(it_j[:], [[1, NJ]], base=0, channel_multiplier=0)
                jf = pp.tile([128, NJ], FP32)
                nc.vector.tensor_copy(jf[:], it_j[:])
                it_p = pp.tile([128, 1], I32)
                nc.gpsimd.iota(it_p[:], [[0, 1]], base=0, channel_multiplier=1)
                pf = pp.tile([128, 1], FP32)
                nc.vector.tensor_copy(pf[:], it_p[:])
                it_ky = pp.tile([128, 3, 3, NJ], I32)
                nc.gpsimd.iota(
                    it_ky[:], [[1, 3], [0, 3], [0, NJ]], base=0, channel_multiplier=0
                )
                kyf = pp.tile(shp, FP32, tag="kyf")
                nc.vector.tensor_copy(
                    kyf[:, :, :].rearrange("p (a b) j -> p a b j", a=3), it_ky[:]
                )
                it_kx = pp.tile([128, 3, 3, NJ], I32)
                nc.gpsimd.iota(
                    it_kx[:], [[0, 3], [1, 3], [0, NJ]], base=0, channel_multiplier=0
                )
                kxf = pp.tile(shp, FP32, tag="kxf")
                nc.vector.tensor_copy(
                    kxf[:, :, :].rearrange("p (a b) j -> p a b j", a=3), it_kx[:]
                )

                # ho = 2*j + p//64 ; wo = p%64  (token t = j*128 + p)
                t1 = pp.tile([128, 1], FP32, tag="t1")
                nc.vector.tensor_scalar(t1[:], pf[:], 1.0 / 64.0, None, A.mult)
                t2 = pp.tile([128, 1], FP32, tag="t2")
                nc.vector.tensor_scalar(t2[:], t1[:], 8388608.0, 8388608.0, A.add, A.subtract)
                p64 = pp.tile([128, 1], FP32, tag="p64")
                nc.vector.tensor_tensor(p64[:], t2[:], t1[:], A.is_gt)
                nc.vector.tensor_tensor(p64[:], t2[:], p64[:], A.subtract)
                wo = pp.tile([128, 1], FP32, tag="wo")
                nc.vector.tensor_scalar(wo[:], p64[:], -64.0, None, A.mult)
                nc.vector.tensor_tensor(wo[:], wo[:], pf[:], A.add)
                ho = pp.tile([128, NJ], FP32, tag="ho")
                nc.vector.tensor_scalar(ho[:], jf[:], 2.0, p64[:, 0:1], A.mult, A.add)

                # biased sample coords: pyb = oy + ky + ho + 7  (bias +8, base -1)
                pyb = f32("pyb")
                nc.vector.tensor_tensor(pyb[:], oy, kyf[:], A.add)
                nc.vector.scalar_tensor_tensor(
                    pyb[:], pyb[:], 7.0, ho[:].unsqueeze(1).broadcast_to(shp), A.add, A.add
                )
                pxb = f32("pxb")
                nc.vector.tensor_tensor(pxb[:], ox, kxf[:], A.add)
                nc.vector.scalar_tensor_tensor(
                    pxb[:], pxb[:], 7.0, wo[:].unsqueeze(1).broadcast_to(shp), A.add, A.add
                )

                fy = f32("fy")
                y0b = f32("y0b")
                nc.vector.tensor_scalar(fy[:], pyb[:], 8388608.0, 8388608.0, A.add, A.subtract)
                nc.vector.tensor_tensor(y0b[:], fy[:], pyb[:], A.is_gt)
                nc.vector.tensor_tensor(y0b[:], fy[:], y0b[:], A.subtract)
                nc.vector.tensor_tensor(fy[:], pyb[:], y0b[:], A.subtract)
                fx = f32("fx")
                x0b = f32("x0b")
                nc.vector.tensor_scalar(fx[:], pxb[:], 8388608.0, 8388608.0, A.add, A.subtract)
                nc.vector.tensor_tensor(x0b[:], fx[:], pxb[:], A.is_gt)
                nc.vector.tensor_tensor(x0b[:], fx[:], x0b[:], A.subtract)
                nc.vector.tensor_tensor(fx[:], pxb[:], x0b[:], A.subtract)

                ta = f32("ta")
                tb = f32("tb")
                # gather superpixel index:
                # idx = (clip(y0b,7,71)-7)*66 + clip(x0b,7,71)-7
                nc.vector.tensor_scalar(ta[:], y0b[:], 7.0, 71.0, A.max, A.min)
                nc.vector.tensor_scalar(tb[:], x0b[:], 7.0, 71.0, A.max, A.min)
                tc_ = f32("tc_")
                nc.vector.scalar_tensor_tensor(tc_[:], ta[:], 66.0, tb[:], A.mult, A.add)
                nc.vector.tensor_scalar(tc_[:], tc_[:], 469.0, None, A.subtract)
                idx16 = pp.tile([128, K, NJ], I16)
                nc.vector.tensor_copy(idx16[:], tc_[:])

                # token-major [p,(k,j)] -> p-major HBM staging
                nc.sync.dma_start(out=ilin_d[:, :], in_=idx16[:])
                # wrapped-16 load into group 0: idx for (k, c) lives at
                # p = (16c + p16) % 128, j = (16c + p16) // 128, decomposed
                # affine as c = 8*ch + cl
                nc.sync.dma_start(
                    out=idx_w[0:16, :, :].rearrange(
                        "p k (ch cl) -> p k ch cl", cl=8
                    ),
                    in_=AP(ilin_d, 0, [[288, 16], [32, K], [1, 32], [16 * 288, 8]]),
                )
                for g in range(1, 8):
                    nc.sync.dma_start(
                        out=idx_w[16 * g : 16 * (g + 1), :, :],
                        in_=idx_w[0:16, :, :],
                    )

                # validity-masked 1D weights:
                # wy0 = (1-fy)*[8<=y0b<=71], wy1 = fy*[7<=y0b<=70]
                nc.vector.tensor_scalar(ta[:], y0b[:], 8.0, None, A.is_ge)
                nc.vector.tensor_scalar(tb[:], y0b[:], 71.0, None, A.is_le)
                vy0 = f32("vy0")
                nc.vector.tensor_tensor(vy0[:], ta[:], tb[:], A.mult)
                nc.vector.tensor_scalar(ta[:], y0b[:], 7.0, None, A.is_ge)
                nc.vector.tensor_scalar(tb[:], y0b[:], 70.0, None, A.is_le)
                vy1 = f32("vy1")
                nc.vector.tensor_tensor(vy1[:], ta[:], tb[:], A.mult)
                u0 = f32("u0")
                nc.vector.tensor_scalar(u0[:], fy[:], -1.0, 1.0, A.mult, A.add)
                wy0 = f32("wy0")
                nc.vector.tensor_tensor(wy0[:], u0[:], vy0[:], A.mult)
                wy1 = f32("wy1")
                nc.vector.tensor_tensor(wy1[:], fy[:], vy1[:], A.mult)
                # wx0 = (1-fx)*[8<=x0b<=71], wx1 = fx*[7<=x0b<=70]
                nc.vector.tensor_scalar(ta[:], x0b[:], 8.0, None, A.is_ge)
                nc.vector.tensor_scalar(tb[:], x0b[:], 71.0, None, A.is_le)
                vx0 = f32("vx0")
                nc.vector.tensor_tensor(vx0[:], ta[:], tb[:], A.mult)
                nc.vector.tensor_scalar(ta[:], x0b[:], 7.0, None, A.is_ge)
                nc.vector.tensor_scalar(tb[:], x0b[:], 70.0, None, A.is_le)
                vx1 = f32("vx1")
                nc.vector.tensor_tensor(vx1[:], ta[:], tb[:], A.mult)
                nc.vector.tensor_scalar(u0[:], fx[:], -1.0, 1.0, A.mult, A.add)
                wx0 = f32("wx0")
                nc.vector.tensor_tensor(wx0[:], u0[:], vx0[:], A.mult)
                wx1 = f32("wx1")
                nc.vector.tensor_tensor(wx1[:], fx[:], vx1[:], A.mult)

                # corner weight products; corner order in the gathered
                # element: [A0 B0 A1 B1] = (x-slot outer, y-row inner)
                for dup in range(2):
                    nc.vector.tensor_tensor(
                        wpack[:, :, :, 0, 0, dup], wy0[:], wx0[:], A.mult
                    )
                    nc.vector.tensor_tensor(
                        wpack[:, :, :, 0, 1, dup], wy1[:], wx0[:], A.mult
                    )
                    nc.vector.tensor_tensor(
                        wpack[:, :, :, 1, 0, dup], wy0[:], wx1[:], A.mult
                    )
                    nc.vector.tensor_tensor(
                        wpack[:, :, :, 1, 1, dup], wy1[:], wx1[:], A.mult
                    )

                # ---- x -> channels-last superpixel image in HBM ----
                xcl_sb = pp.tile([128, HW // 128, C], BF16)
                # interior write AP dims: (ql 64, s_loc 8, c 256) per qh half,
                # matching the sbuf source [p=(qh ql), s_loc, c]; pixel
                # t = s*128+q, prow = 2s+qh (+16 per q4 group), pcol = ql.
                wdims = [
                    [2 * C, 64],
                    [2 * 132 * C, 8],
                    [1, C],
                ]
                for s2 in range(HW // 256):
                    pt = pps.tile([128, 2, 2, 128], BF16, tag="pt")
                    for sl in range(2):
                        for ch in range(2):
                            nc.tensor.transpose(
                                pt[:, sl, ch, :],
                                x_bf[:, ch, ts(2 * s2 + sl, 128)],
                                ident[:],
                            )
                    nc.scalar.copy(
                        xcl_sb[:, 2 * s2 : 2 * s2 + 2, :].rearrange(
                            "p s c -> p (s c)"
                        ),
                        pt[:].rearrange("p a b c -> p (a b c)"),
                    )
                    s = 2 * s2 + 1
                    if s % 8 == 7:
                        q4 = s // 8
                        for qh in range(2):
                            src = xcl_sb[
                                64 * qh : 64 * (qh + 1), q4 * 8 : (q4 + 1) * 8, :
                            ]
                            base = (q4 * 16 + qh) * 132 * C
                            # split across SP (HWDGE) and Pool (SWDGE) queues
                            # so the writes don't serialize behind the idx
                            # staging chain on SP
                            eng = nc.sync if qh == 0 else nc.gpsimd
                            # slot 0: pixel row prow at store-row prow+1
                            eng.dma_start(
                                out=AP(iz_d, base + 134 * C, wdims), in_=src
                            )
                            # slot 1: pixel row prow at store-row prow
                            eng.dma_start(
                                out=AP(iz_d, base + 3 * C, wdims), in_=src
                            )

                # ---- weight lhsT (SWDGE load with fp32->bf16 cast) ----
                w_bf = pp.tile([128, 2, K * C], BF16)
                nc.gpsimd.dma_start(
                    out=w_bf[:], in_=w_in[:, :].rearrange("(h p) c -> p h c", p=128)
                )
                for kc2 in range(2 * K):
                    ptw = pps.tile([128, 2, 128], BF16, tag="pt")
                    for oh in range(2):
                        nc.tensor.transpose(
                            ptw[:, oh, :], w_bf[:, oh, ts(kc2, 128)], ident[:]
                        )
                    nc.scalar.copy(wT[:, kc2, :], ptw[:])


            # ---------------- main loop ----------------
            # gather source: start positions at superpixel granularity
            xsrc = AP(iz_d, 0, [[SP_ELEMS, NROW * NCOL - 1], [1, 2 * SP_ELEMS]])
            NJC = NCH // 128  # 8 j-columns per chunk
            nreg = nc.gpsimd.to_reg(NCH)

            with (
                tc.tile_pool(name="vp", bufs=5) as vp,
                tc.tile_pool(name="cc", bufs=6) as ccp,
                tc.tile_pool(name="ob", bufs=2) as obp,
                tc.tile_pool(name="mps", bufs=1, space="PSUM") as mps,
                tc.tile_pool(name="cps", bufs=2, space="PSUM") as cps,
            ):
                iters = [(nch, k) for nch in range(NCHUNKS) for k in range(K)]
                vts = {}

                def issue_gather(i):
                    nch_, k_ = iters[i]
                    v = vp.tile([128, NJC, 4 * C], BF16, tag="v", name="v")
                    if os.environ.get("KBISECT") == "1":
                        nc.vector.memset(v[:], 0.25)
                    else:
                        nc.gpsimd.dma_gather(
                            out_ap=v[:],
                            in_ap=xsrc,
                            idxs_ap=idx_w[
                                :,
                                k_,
                                nch_ * (NCH // 16) : (nch_ + 1) * (NCH // 16),
                            ],
                            num_idxs=NCH,
                            num_idxs_reg=nreg,
                            elem_size=2 * SP_ELEMS,
                            elem_step=SP_ELEMS,
                        )
                    vts[i] = v

                issue_gather(0)
                issue_gather(1)
                issue_gather(2)
                issue_gather(3)
                ps = None
                for it in range(len(iters)):
                    nch, k = iters[it]
                    if it + 4 < len(iters):
                        issue_gather(it + 4)
                    if k == 0:
                        ps = [
                            [mps.tile([128, 512], FP32, tag=f"ps{oh}{n2}", name=f"ps{oh}{n2}") for n2 in range(2)]
                            for oh in range(2)
                        ]
                    v = vts.pop(it)
                    # blend: v *= wpack (per-token corner weights), then
                    # 4-corner tree sum (second level in-place into v)
                    v6 = v[:, :, :].rearrange(
                        "p j (x y h two) -> p j x y h two", x=2, y=2, two=2
                    )
                    w6 = (
                        wpack[:, k, nch * NJC : (nch + 1) * NJC, :, :, :]
                        .unsqueeze(4)
                        .broadcast_to([128, NJC, 2, 2, C // 2, 2])
                    )
                    v4 = v[:, :, :].rearrange("p j (c4 c) -> p j c4 c", c4=4)
                    ct = ccp.tile([128, NJC, C], BF16, tag="ct")
                    if os.environ.get("KBISECT") == "3":
                        nc.vector.tensor_copy(ct[:], v4[:, :, 0])
                    else:
                        nc.vector.tensor_tensor(v6, v6, w6, A.mult)
                        nc.vector.tensor_tensor(
                            v4[:, :, 0:2], v4[:, :, 0:2], v4[:, :, 2:4], A.add
                        )
                        # final pair-sum on GPSIMD: Pool engine has headroom
                        # while DVE is the main-loop rate limiter
                        nc.gpsimd.tensor_tensor(
                            ct[:], v4[:, :, 0], v4[:, :, 1], A.add
                        )

                    # transpose to channel-major via PE block transposes
                    # (keeps the SDMA engines free for the main gather) and
                    # one ACT PSUM->SBUF copy
                    cm = ccp.tile([128, 2, NCH], BF16, tag="cm")
                    if os.environ.get("KBISECT") in ("1", "2"):
                        nc.vector.memset(cm[:], 0.125)
                    else:
                        cm_ps = cps.tile([128, 2, NJC, 128], BF16, tag="cmps")
                        for ch in range(2):
                            for j in range(NJC):
                                nc.tensor.transpose(
                                    cm_ps[:, ch, j, :],
                                    ct[:, j, ts(ch, 128)],
                                    ident[:],
                                )
                        nc.scalar.copy(cm[:], cm_ps[:])

                    for oh in range(2):
                        for ch in range(2):
                            lhsT = wT[:, 2 * k + ch, ts(oh, 128)]
                            for n2 in range(2):
                                nc.tensor.matmul(
                                    ps[oh][n2][:],
                                    lhsT,
                                    cm[:, ch, ts(n2, 512)],
                                    start=(k == 0 and ch == 0),
                                    stop=(k == K - 1 and ch == 1),
                                )

                    if k != K - 1:
                        continue
                    ob = obp.tile([128, 2, NCH], FP32, tag="ob")
                    for oh in range(2):
                        for n2 in range(2):
                            nc.scalar.copy(ob[:, oh, ts(n2, 512)], ps[oh][n2][:])
                    for oh in range(2):
                        for n2 in range(2):
                            nc.sync.dma_start(
                                out=out_d[
                                    128 * oh : 128 * (oh + 1),
                                    nch * NCH + 512 * n2 : nch * NCH + 512 * (n2 + 1),
                                ],
                                in_=ob[:, oh, ts(n2, 512)],
                            )
    nc.compile()
    return nc


_NC_CACHE = None


def _get_nc():
    global _NC_CACHE
    if _NC_CACHE is None:
        _NC_CACHE = build_nc()
    return _NC_CACHE


def kernel(x: np.ndarray, offset: np.ndarray, weight: np.ndarray) -> np.ndarray:
    return _run(x, offset, weight)[0]


def _run(x, offset, weight, **spmd_kwargs):
    assert x.shape == (B, C, H, W) and offset.shape == (B, 2 * K, H, W)
    nc = _get_nc()
    # [cout, cin, 3, 3] -> [cout, (k, c)]
    w_perm = np.ascontiguousarray(
        weight.reshape(COUT, C, K).transpose(0, 2, 1).reshape(COUT, K * C)
    ).astype(np.float32)
    in_maps = [
        {
            "x": np.ascontiguousarray(x[b].reshape(C, HW)).astype(np.float32),
            "off": np.ascontiguousarray(offset[b].reshape(2 * K, HW)).astype(
                np.float32
            ),
            "w": w_perm,
        }
        for b in range(B)
    ]
    res = run_bass_kernel_spmd(nc, in_maps, core_ids=list(range(B)), **spmd_kwargs)
    out = np.stack([res.results[b]["out"].reshape(COUT, H, W) for b in range(B)])
    return out.astype(np.float32), res


if __name__ == "__main__":
    d = np.load("/root/problem/inputs.npz")
    out = kernel(d["x"], d["offset"], d["weight"])
    ref = np.load("/root/problem/ref_out_np.npy")
    err = np.abs(out - ref).max()
    rel = err / np.abs(ref).max()
    print("absmax err:", err, "rel:", rel)


# revision 56
# speedup vs baseline: 1.0003x; 1.0003x over previous
"""Deformable conv (3x3, pad=1, B=8, Cin=Cout=256, H=W=64) on 8 TRN2 NeuronCores.

Strategy (data-parallel over batch, one image per core):
  1. Prologue per core: build a zero-padded *superpixel* image IZ in HBM via PE
     transpose: IZ[R, X] = [pix(R-1, X-1), pix(R, X-1)] (vertical pixel pair,
     512 ch contiguous, bf16), R in 0..64, X in 0..65, with zero rows/cols on
     the borders.  One 2KB gather descriptor starting at superpixel (R, X)
     then fetches all 4 bilinear corners [A0 B0 A1 B1] of a tap sample.
     Also: transpose the (k,c)-ordered weight matrix into lhsT layout; compute
     per-token superpixel indices + 4 corner weights on DVE from the offsets.
  2. Main loop over (token-chunk, tap): ONE SWDGE dma_gather fetches the 4
     corners per output token (token-major: token on partition, 4x256 channels
     on free dim).  DVE multiplies the corners by per-token weights (free-dim
     step-0 broadcast APs with dup-pairs for the 2x_1p perf mode) and does the
     first pair-sum; GPSIMD does the second pair-sum.  PE block-transposes the
     blended tile to channel-major [c, token] in PSUM (ACT copies it back to
     SBUF), keeping the SDMA engines free for the main gather.  TensorE
     accumulates W_k^T @ cols into PSUM over all 9 taps x 2 c-halves.
  3. PSUM -> SBUF -> HBM fp32 output.

Boundary handling is entirely via the zero padding: the gather index is
(clip(y0,-1,63)+1)*66 + clip(x0,-1,63)+1 and out-of-image corners get weight 0
through the validity masks (exactly the reference's `valid` mask).
"""

import os

import numpy as np

import concourse.bacc as bacc
import concourse.bass as bass
import concourse.mybir as mybir
from concourse.bass import AP, ts
from concourse.bass_utils import run_bass_kernel_spmd
from concourse.masks import make_identity
from concourse.tile import TileContext

FP32 = mybir.dt.float32
BF16 = mybir.dt.bfloat16
I16 = mybir.dt.int16
I32 = mybir.dt.int32

B = 8
C = 256
H = W = 64
HW = H * W           # 4096 pixels / tokens per tap
K = 9                # 3x3 taps
COUT = 256
NCH = 1024           # tokens per chunk
NCHUNKS = HW // NCH  # 4
A = mybir.AluOpType

# superpixel table geometry: 65 store-rows (R=0..64), 66 store-cols (X=0..65),
# 2 pixel slots of 256 ch each.  flat [65*66*2, 256] bf16.
NROW = 65
NCOL = 66
SP_ELEMS = 2 * C                 # 512 elements per superpixel
IZ_ROWS = NROW * NCOL * 2        # 8580 rows of 256
ROW_STRIDE = NCOL * SP_ELEMS     # 33792 elems per store-row


def build_nc() -> bass.Bass:
    nc = bacc.Bacc(target_bir_lowering=False)

    x_in = nc.dram_tensor("x", [C, HW], FP32, kind="ExternalInput")
    off_in = nc.dram_tensor("off", [2 * K, HW], FP32, kind="ExternalInput")
    # host pre-permutes weight to [cout, (k, c)] layout
    w_in = nc.dram_tensor("w", [COUT, K * C], FP32, kind="ExternalInput")
    out_d = nc.dram_tensor("out", [COUT, HW], FP32, kind="ExternalOutput")

    # zero-padded superpixel image (see module docstring)
    iz_d = nc.dram_tensor("iz", [IZ_ROWS, C], BF16, kind="Internal")
    # token-major -> wrapped-16 index staging
    ilin_d = nc.dram_tensor("ilin", [128, K * HW // 128], I16, kind="Internal")

    with TileContext(nc) as tc:
        with tc.tile_pool(name="const", bufs=1) as cp:
            ident = cp.tile([128, 128], BF16)
            make_identity(nc, ident)
            identf = cp.tile([128, 128], FP32)
            make_identity(nc, identf)
            # lhsT tiles: block kc2 = k*2 + chalf holds rows c (128) x cout (256)
            wT = cp.tile([128, 2 * K, 256], BF16)
            # corner weights per token: (k, j, xpos, yrow, dup); bf16 dup
            # pairs give the blend's broadcast operand a stride-1 innermost
            # dim (enables the DVE 2x_1p perf mode)
            wpack = cp.tile([128, K, HW // 128, 2, 2, 2], BF16)
            # wrapped-16 gather indices per tap
            idx_w = cp.tile([128, K, HW // 16], I16)

            with (
                tc.tile_pool(name="prep", bufs=1) as pp,
                tc.tile_pool(name="ppsum", bufs=4, space="PSUM") as pps,
            ):
                # ---- x -> channels-last bf16 superpixel image in HBM ----
                x_bf = pp.tile([128, 2, HW], BF16)
                nc.gpsimd.dma_start(
                    out=x_bf[:], in_=x_in[:, :].rearrange("(h p) c -> p h c", p=128)
                )
                # zero borders first (rows -1/64 and cols -1/64 of the pad)
                zpad = pp.tile([128, 2 * C], BF16)
                nc.vector.memset(zpad[:], 0.0)
                # store-row 0, slot 0 (pixel row -1)
                nc.sync.dma_start(
                    out=AP(iz_d, 0, [[SP_ELEMS, NCOL], [1, C]]),
                    in_=zpad[0:NCOL, 0:C],
                )
                # store-row 64, slot 1 (pixel row 64)
                nc.sync.dma_start(
                    out=AP(iz_d, 64 * ROW_STRIDE + C, [[SP_ELEMS, NCOL], [1, C]]),
                    in_=zpad[0:NCOL, 0:C],
                )
                # store-col 0 (pixel col -1), both slots
                nc.sync.dma_start(
                    out=AP(iz_d, 0, [[ROW_STRIDE, NROW], [1, SP_ELEMS]]),
                    in_=zpad[0:NROW, :],
                )
                # store-col 65 (pixel col 64), both slots
                nc.sync.dma_start(
                    out=AP(iz_d, 65 * SP_ELEMS, [[ROW_STRIDE, NROW], [1, SP_ELEMS]]),
                    in_=zpad[0:NROW, :],
                )
                # ---- offsets -> indices + corner weights ----
                # (before the image transposes: the off->idx chain is the
                # critical path to the first gather; its 8 small PE transposes
                # go ahead of the 64 x-transposes, and the xcl PSUM->SBUF
                # copies run on ACT so DVE is free for this chain)
                # natural load (18 contiguous 16KB descriptors), then PE
                # transpose to token-major [128, j, r]
                off_nat = pp.tile([128, HW], FP32)
                nc.sync.dma_start(out=off_nat[0 : 2 * K, :], in_=off_in[:, :])
                off_t2 = pp.tile([128, HW // 128, 2 * K], FP32)
                for s4 in range(HW // 128 // 4):
                    ptf = pps.tile([128, 4, 2 * K], FP32, tag="ptf")
                    for i in range(4):
                        nc.tensor.transpose(
                            ptf[:, i, :],
                            off_nat[0 : 2 * K, ts(4 * s4 + i, 128)],
                            identf[0 : 2 * K, 0 : 2 * K],
                        )
                    nc.vector.tensor_copy(off_t2[:, 4 * s4 : 4 * s4 + 4, :], ptf[:])
                off_r = off_t2[:, :, :].rearrange("p j (k s) -> p s k j", s=2)
                oy = off_r[:, 0]  # [128, 9, 32] (k, j) strides (2, 18)
                ox = off_r[:, 1]

                NJ = HW // 128  # 32
                shp = [128, K, NJ]

                def f32(tag):
                    return pp.tile(shp, FP32, tag=tag, name=tag)

                # iotas
                it_j = pp.tile([128, NJ], I32)
                nc.gpsimd.iota(it_j[:], [[1, NJ]], base=0, channel_multiplier=0)
                jf = pp.tile([128, NJ], FP32)
                nc.vector.tensor_copy(jf[:], it_j[:])
                it_p = pp.tile([128, 1], I32)
                nc.gpsimd.iota(it_p[:], [[0, 1]], base=0, channel_multiplier=1)
                pf = pp.tile([128, 1], FP32)
                nc.vector.tensor_copy(pf[:], it_p[:])
                it_ky = pp.tile([128, 3, 3, NJ], I32)
                nc.gpsimd.iota(
                    it_ky[:], [[1, 3], [0, 3], [0, NJ]], base=0, channel_multiplier=0
                )
                kyf = pp.tile(shp, FP32, tag="kyf")
                nc.vector.tensor_copy(
                    kyf[:, :, :].rearrange("p (a b) j -> p a b j", a=3), it_ky[:]
                )
                it_kx = pp.tile([128, 3, 3, NJ], I32)
                nc.gpsimd.iota(
                    it_kx[:], [[0, 3], [1, 3], [0, NJ]], base=0, channel_multiplier=0
                )
                kxf = pp.tile(shp, FP32, tag="kxf")
                nc.vector.tensor_copy(
                    kxf[:, :, :].rearrange("p (a b) j -> p a b j", a=3), it_kx[:]
                )

                # ho = 2*j + p//64 ; wo = p%64  (token t = j*128 + p)
                t1 = pp.tile([128, 1], FP32, tag="t1")
                nc.vector.tensor_scalar(t1[:], pf[:], 1.0 / 64.0, None, A.mult)
                t2 = pp.tile([128, 1], FP32, tag="t2")
                nc.vector.tensor_scalar(t2[:], t1[:], 8388608.0, 8388608.0, A.add, A.subtract)
                p64 = pp.tile([128, 1], FP32, tag="p64")
                nc.vector.tensor_tensor(p64[:], t2[:], t1[:], A.is_gt)
                nc.vector.tensor_tensor(p64[:], t2[:], p64[:], A.subtract)
                wo = pp.tile([128, 1], FP32, tag="wo")
                nc.vector.tensor_scalar(wo[:], p64[:], -64.0, None, A.mult)
                nc.vector.tensor_tensor(wo[:], wo[:], pf[:], A.add)
                ho = pp.tile([128, NJ], FP32, tag="ho")
                nc.vector.tensor_scalar(ho[:], jf[:], 2.0, p64[:, 0:1], A.mult, A.add)

                # biased sample coords: pyb = oy + ky + ho + 7  (bias +8, base -1)
                pyb = f32("pyb")
                nc.vector.tensor_tensor(pyb[:], oy, kyf[:], A.add)
                nc.vector.scalar_tensor_tensor(
                    pyb[:], pyb[:], 7.0, ho[:].unsqueeze(1).broadcast_to(shp), A.add, A.add
                )
                pxb = f32("pxb")
                nc.vector.tensor_tensor(pxb[:], ox, kxf[:], A.add)
                nc.vector.scalar_tensor_tensor(
                    pxb[:], pxb[:], 7.0, wo[:].unsqueeze(1).broadcast_to(shp), A.add, A.add
                )

                fy = f32("fy")
                y0b = f32("y0b")
                nc.vector.tensor_scalar(fy[:], pyb[:], 8388608.0, 8388608.0, A.add, A.subtract)
                nc.vector.tensor_tensor(y0b[:], fy[:], pyb[:], A.is_gt)
                nc.vector.tensor_tensor(y0b[:], fy[:], y0b[:], A.subtract)
                nc.vector.tensor_tensor(fy[:], pyb[:], y0b[:], A.subtract)
                fx = f32("fx")
                x0b = f32("x0b")
                nc.vector.tensor_scalar(fx[:], pxb[:], 8388608.0, 8388608.0, A.add, A.subtract)
                nc.vector.tensor_tensor(x0b[:], fx[:], pxb[:], A.is_gt)
                nc.vector.tensor_tensor(x0b[:], fx[:], x0b[:], A.subtract)
                nc.vector.tensor_tensor(fx[:], pxb[:], x0b[:], A.subtract)

                ta = f32("ta")
                tb = f32("tb")
                # gather superpixel index:
                # idx = (clip(y0b,7,71)-7)*66 + clip(x0b,7,71)-7
                nc.vector.tensor_scalar(ta[:], y0b[:], 7.0, 71.0, A.max, A.min)
                nc.vector.tensor_scalar(tb[:], x0b[:], 7.0, 71.0, A.max, A.min)
                tc_ = f32("tc_")
                nc.vector.scalar_tensor_tensor(tc_[:], ta[:], 66.0, tb[:], A.mult, A.add)
                nc.vector.tensor_scalar(tc_[:], tc_[:], 469.0, None, A.subtract)
                idx16 = pp.tile([128, K, NJ], I16)
                nc.vector.tensor_copy(idx16[:], tc_[:])

                # token-major [p,(k,j)] -> p-major HBM staging
                nc.sync.dma_start(out=ilin_d[:, :], in_=idx16[:])
                # wrapped-16 load into group 0: idx for (k, c) lives at
                # p = (16c + p16) % 128, j = (16c + p16) // 128, decomposed
                # affine as c = 8*ch + cl
                nc.sync.dma_start(
                    out=idx_w[0:16, :, :].rearrange(
                        "p k (ch cl) -> p k ch cl", cl=8
                    ),
                    in_=AP(ilin_d, 0, [[288, 16], [32, K], [1, 32], [16 * 288, 8]]),
                )
                for g in range(1, 8):
                    nc.sync.dma_start(
                        out=idx_w[16 * g : 16 * (g + 1), :, :],
                        in_=idx_w[0:16, :, :],
                    )

                # validity-masked 1D weights:
                # wy0 = (1-fy)*[8<=y0b<=71], wy1 = fy*[7<=y0b<=70]
                nc.vector.tensor_scalar(ta[:], y0b[:], 8.0, None, A.is_ge)
                nc.vector.tensor_scalar(tb[:], y0b[:], 71.0, None, A.is_le)
                vy0 = f32("vy0")
                nc.vector.tensor_tensor(vy0[:], ta[:], tb[:], A.mult)
                nc.vector.tensor_scalar(ta[:], y0b[:], 7.0, None, A.is_ge)
                nc.vector.tensor_scalar(tb[:], y0b[:], 70.0, None, A.is_le)
                vy1 = f32("vy1")
                nc.vector.tensor_tensor(vy1[:], ta[:], tb[:], A.mult)
                u0 = f32("u0")
                nc.vector.tensor_scalar(u0[:], fy[:], -1.0, 1.0, A.mult, A.add)
                wy0 = f32("wy0")
                nc.vector.tensor_tensor(wy0[:], u0[:], vy0[:], A.mult)
                wy1 = f32("wy1")
                nc.vector.tensor_tensor(wy1[:], fy[:], vy1[:], A.mult)
                # wx0 = (1-fx)*[8<=x0b<=71], wx1 = fx*[7<=x0b<=70]
                nc.vector.tensor_scalar(ta[:], x0b[:], 8.0, None, A.is_ge)
                nc.vector.tensor_scalar(tb[:], x0b[:], 71.0, None, A.is_le)
                vx0 = f32("vx0")
                nc.vector.tensor_tensor(vx0[:], ta[:], tb[:], A.mult)
                nc.vector.tensor_scalar(ta[:], x0b[:], 7.0, None, A.is_ge)
                nc.vector.tensor_scalar(tb[:], x0b[:], 70.0, None, A.is_le)
                vx1 = f32("vx1")
                nc.vector.tensor_tensor(vx1[:], ta[:], tb[:], A.mult)
                nc.vector.tensor_scalar(u0[:], fx[:], -1.0, 1.0, A.mult, A.add)
                wx0 = f32("wx0")
                nc.vector.tensor_tensor(wx0[:], u0[:], vx0[:], A.mult)
                wx1 = f32("wx1")
                nc.vector.tensor_tensor(wx1[:], fx[:], vx1[:], A.mult)

                # corner weight products; corner order in the gathered
                # element: [A0 B0 A1 B1] = (x-slot outer, y-row inner)
                for dup in range(2):
                    nc.vector.tensor_tensor(
                        wpack[:, :, :, 0, 0, dup], wy0[:], wx0[:], A.mult
                    )
                    nc.vector.tensor_tensor(
                        wpack[:, :, :, 0, 1, dup], wy1[:], wx0[:], A.mult
                    )
                    nc.vector.tensor_tensor(
                        wpack[:, :, :, 1, 0, dup], wy0[:], wx1[:], A.mult
                    )
                    nc.vector.tensor_tensor(
                        wpack[:, :, :, 1, 1, dup], wy1[:], wx1[:], A.mult
                    )

                # ---- x -> channels-last superpixel image in HBM ----
                xcl_sb = pp.tile([128, HW // 128, C], BF16)
                # interior write AP dims: (ql 64, s_loc 8, c 256) per qh half,
                # matching the sbuf source [p=(qh ql), s_loc, c]; pixel
                # t = s*128+q, prow = 2s+qh (+16 per q4 group), pcol = ql.
                wdims = [
                    [2 * C, 64],
                    [2 * 132 * C, 8],
                    [1, C],
                ]
                for s in range(HW // 128):
                    pt = pps.tile([128, 2, 128], BF16, tag="pt")
                    for ch in range(2):
                        nc.tensor.transpose(
                            pt[:, ch, :], x_bf[:, ch, ts(s, 128)], ident[:]
                        )
                    nc.scalar.copy(xcl_sb[:, s, :], pt[:])
                    if s % 8 == 7:
                        q4 = s // 8
                        for qh in range(2):
                            src = xcl_sb[
                                64 * qh : 64 * (qh + 1), q4 * 8 : (q4 + 1) * 8, :
                            ]
                            base = (q4 * 16 + qh) * 132 * C
                            # split across SP (HWDGE) and Pool (SWDGE) queues
                            # so the writes don't serialize behind the idx
                            # staging chain on SP
                            eng = nc.sync if qh == 0 else nc.gpsimd
                            # slot 0: pixel row prow at store-row prow+1
                            eng.dma_start(
                                out=AP(iz_d, base + 134 * C, wdims), in_=src
                            )
                            # slot 1: pixel row prow at store-row prow
                            eng.dma_start(
                                out=AP(iz_d, base + 3 * C, wdims), in_=src
                            )

                # ---- weight lhsT (SWDGE load with fp32->bf16 cast) ----
                w_bf = pp.tile([128, 2, K * C], BF16)
                nc.gpsimd.dma_start(
                    out=w_bf[:], in_=w_in[:, :].rearrange("(h p) c -> p h c", p=128)
                )
                for kc2 in range(2 * K):
                    ptw = pps.tile([128, 2, 128], BF16, tag="pt")
                    for oh in range(2):
                        nc.tensor.transpose(
                            ptw[:, oh, :], w_bf[:, oh, ts(kc2, 128)], ident[:]
                        )
                    nc.scalar.copy(wT[:, kc2, :], ptw[:])


            # ---------------- main loop ----------------
            # gather source: start positions at superpixel granularity
            xsrc = AP(iz_d, 0, [[SP_ELEMS, NROW * NCOL - 1], [1, 2 * SP_ELEMS]])
            NJC = NCH // 128  # 8 j-columns per chunk
            nreg = nc.gpsimd.to_reg(NCH)

            with (
                tc.tile_pool(name="vp", bufs=5) as vp,
                tc.tile_pool(name="cc", bufs=6) as ccp,
                tc.tile_pool(name="ob", bufs=2) as obp,
                tc.tile_pool(name="mps", bufs=1, space="PSUM") as mps,
                tc.tile_pool(name="cps", bufs=2, space="PSUM") as cps,
            ):
                iters = [(nch, k) for nch in range(NCHUNKS) for k in range(K)]
                vts = {}

                def issue_gather(i):
                    nch_, k_ = iters[i]
                    v = vp.tile([128, NJC, 4 * C], BF16, tag="v", name="v")
                    if os.environ.get("KBISECT") == "1":
                        nc.vector.memset(v[:], 0.25)
                    else:
                        nc.gpsimd.dma_gather(
                            out_ap=v[:],
                            in_ap=xsrc,
                            idxs_ap=idx_w[
                                :,
                                k_,
                                nch_ * (NCH // 16) : (nch_ + 1) * (NCH // 16),
                            ],
                            num_idxs=NCH,
                            num_idxs_reg=nreg,
                            elem_size=2 * SP_ELEMS,
                            elem_step=SP_ELEMS,
                        )
                    vts[i] = v

                issue_gather(0)
                issue_gather(1)
                issue_gather(2)
                issue_gather(3)
                ps = None
                for it in range(len(iters)):
                    nch, k = iters[it]
                    if it + 4 < len(iters):
                        issue_gather(it + 4)
                    if k == 0:
                        ps = [
                            [mps.tile([128, 512], FP32, tag=f"ps{oh}{n2}", name=f"ps{oh}{n2}") for n2 in range(2)]
                            for oh in range(2)
                        ]
                    v = vts.pop(it)
                    # blend: v *= wpack (per-token corner weights), then
                    # 4-corner tree sum (second level in-place into v)
                    v6 = v[:, :, :].rearrange(
                        "p j (x y h two) -> p j x y h two", x=2, y=2, two=2
                    )
                    w6 = (
                        wpack[:, k, nch * NJC : (nch + 1) * NJC, :, :, :]
                        .unsqueeze(4)
                        .broadcast_to([128, NJC, 2, 2, C // 2, 2])
                    )
                    v4 = v[:, :, :].rearrange("p j (c4 c) -> p j c4 c", c4=4)
                    ct = ccp.tile([128, NJC, C], BF16, tag="ct")
                    if os.environ.get("KBISECT") == "3":
                        nc.vector.tensor_copy(ct[:], v4[:, :, 0])
                    else:
                        nc.vector.tensor_tensor(v6, v6, w6, A.mult)
                        nc.vector.tensor_tensor(
                            v4[:, :, 0:2], v4[:, :, 0:2], v4[:, :, 2:4], A.add
                        )
                        # final pair-sum on GPSIMD: Pool engine has headroom
                        # while DVE is the main-loop rate limiter
                        nc.gpsimd.tensor_tensor(
                            ct[:], v4[:, :, 0], v4[:, :, 1], A.add
                        )

                    # transpose to channel-major via PE block transposes
                    # (keeps the SDMA engines free for the main gather) and
                    # one ACT PSUM->SBUF copy
                    cm = ccp.tile([128, 2, NCH], BF16, tag="cm")
                    if os.environ.get("KBISECT") in ("1", "2"):
                        nc.vector.memset(cm[:], 0.125)
                    else:
                        cm_ps = cps.tile([128, 2, NJC, 128], BF16, tag="cmps")
                        for ch in range(2):
                            for j in range(NJC):
                                nc.tensor.transpose(
                                    cm_ps[:, ch, j, :],
                                    ct[:, j, ts(ch, 128)],
                                    ident[:],
                                )
                        nc.scalar.copy(cm[:], cm_ps[:])

                    for oh in range(2):
                        for ch in range(2):
                            lhsT = wT[:, 2 * k + ch, ts(oh, 128)]
                            for n2 in range(2):
                                nc.tensor.matmul(
                                    ps[oh][n2][:],
                                    lhsT,
                                    cm[:, ch, ts(n2, 512)],
                                    start=(k == 0 and ch == 0),
                                    stop=(k == K - 1 and ch == 1),
                                )

                    if k != K - 1:
                        continue
                    ob = obp.tile([128, 2, NCH], FP32, tag="ob")
                    for oh in range(2):
                        for n2 in range(2):
                            nc.scalar.copy(ob[:, oh, ts(n2, 512)], ps[oh][n2][:])
                    for oh in range(2):
                        for n2 in range(2):
                            nc.sync.dma_start(
                                out=out_d[
                                    128 * oh : 128 * (oh + 1),
                                    nch * NCH + 512 * n2 : nch * NCH + 512 * (n2 + 1),
                                ],
                                in_=ob[:, oh, ts(n2, 512)],
                            )
    nc.compile()
    return nc


_NC_CACHE = None


def _get_nc():
    global _NC_CACHE
    if _NC_CACHE is None:
        _NC_CACHE = build_nc()
    return _NC_CACHE


def kernel(x: np.ndarray, offset: np.ndarray, weight: np.ndarray) -> np.ndarray:
    return _run(x, offset, weight)[0]


def _run(x, offset, weight, **spmd_kwargs):
    assert x.shape == (B, C, H, W) and offset.shape == (B, 2 * K, H, W)
    nc = _get_nc()
    # [cout, cin, 3, 3] -> [cout, (k, c)]
    w_perm = np.ascontiguousarray(
        weight.reshape(COUT, C, K).transpose(0, 2, 1).reshape(COUT, K * C)
    ).astype(np.float32)
    in_maps = [
        {
            "x": np.ascontiguousarray(x[b].reshape(C, HW)).astype(np.float32),
            "off": np.ascontiguousarray(offset[b].reshape(2 * K, HW)).astype(
                np.float32
            ),
            "w": w_perm,
        }
        for b in range(B)
    ]
    res = run_bass_kernel_spmd(nc, in_maps, core_ids=list(range(B)), **spmd_kwargs)
    out = np.stack([res.results[b]["out"].reshape(COUT, H, W) for b in range(B)])
    return out.astype(np.float32), res


if __name__ == "__main__":
    d = np.load("/root/problem/inputs.npz")
    out = kernel(d["x"], d["offset"], d["weight"])
    ref = np.load("/root/problem/ref_out_np.npy")
    err = np.abs(out - ref).max()
    rel = err / np.abs(ref).max()
    print("absmax err:", err, "rel:", rel)


# revision 58
# speedup vs baseline: 1.0866x; 1.0863x over previous
"""Deformable conv (3x3, pad=1, B=8, Cin=Cout=256, H=W=64) on 8 TRN2 NeuronCores.

Strategy (data-parallel over batch, one image per core):
  1. Prologue per core: build a zero-padded *superpixel* image IZ in HBM via PE
     transpose: IZ[R, X] = [pix(R-1, X-1), pix(R, X-1)] (vertical pixel pair,
     512 ch contiguous, bf16), R in 0..64, X in 0..65, with zero rows/cols on
     the borders.  One 2KB gather descriptor starting at superpixel (R, X)
     then fetches all 4 bilinear corners [A0 B0 A1 B1] of a tap sample.
     Also: transpose the (k,c)-ordered weight matrix into lhsT layout; compute
     per-token superpixel indices + 4 corner weights on DVE from the offsets.
  2. Main loop over (token-chunk, tap): ONE SWDGE dma_gather fetches the 4
     corners per output token (token-major: token on partition, 4x256 channels
     on free dim).  DVE multiplies the corners by per-token weights (free-dim
     step-0 broadcast APs with dup-pairs for the 2x_1p perf mode) and does the
     first pair-sum; GPSIMD does the second pair-sum.  PE block-transposes the
     blended tile to channel-major [c, token] in PSUM (ACT copies it back to
     SBUF), keeping the SDMA engines free for the main gather.  TensorE
     accumulates W_k^T @ cols into PSUM over all 9 taps x 2 c-halves.
  3. PSUM -> SBUF -> HBM fp32 output.

Boundary handling is entirely via the zero padding: the gather index is
(clip(y0,-1,63)+1)*66 + clip(x0,-1,63)+1 and out-of-image corners get weight 0
through the validity masks (exactly the reference's `valid` mask).
"""

import os

import numpy as np

import concourse.bacc as bacc
import concourse.bass as bass
import concourse.mybir as mybir
from concourse.bass import AP, ts
from concourse.bass_utils import run_bass_kernel_spmd
from concourse.masks import make_identity
from concourse.tile import TileContext

FP32 = mybir.dt.float32
BF16 = mybir.dt.bfloat16
I16 = mybir.dt.int16
I32 = mybir.dt.int32

B = 8
C = 256
H = W = 64
HW = H * W           # 4096 pixels / tokens per tap
K = 9                # 3x3 taps
COUT = 256
NCH = 1024           # tokens per chunk
NCHUNKS = HW // NCH  # 4
A = mybir.AluOpType

# superpixel table geometry: 65 store-rows (R=0..64), 66 store-cols (X=0..65),
# 2 pixel slots of 256 ch each.  flat [65*66*2, 256] bf16.
NROW = 65
NCOL = 66
SP_ELEMS = 2 * C                 # 512 elements per superpixel
IZ_ROWS = NROW * NCOL * 2        # 8580 rows of 256
ROW_STRIDE = NCOL * SP_ELEMS     # 33792 elems per store-row


def build_nc() -> bass.Bass:
    nc = bacc.Bacc(target_bir_lowering=False)

    x_in = nc.dram_tensor("x", [C, HW], FP32, kind="ExternalInput")
    off_in = nc.dram_tensor("off", [2 * K, HW], FP32, kind="ExternalInput")
    # host pre-permutes weight to [cout, (k, c)] layout
    w_in = nc.dram_tensor("w", [COUT, K * C], FP32, kind="ExternalInput")
    out_d = nc.dram_tensor("out", [COUT, HW], FP32, kind="ExternalOutput")

    # zero-padded superpixel image (see module docstring)
    iz_d = nc.dram_tensor("iz", [IZ_ROWS, C], BF16, kind="Internal")
    # token-major -> wrapped-16 index staging
    ilin_d = nc.dram_tensor("ilin", [128, K * HW // 128], I16, kind="Internal")

    with TileContext(nc) as tc:
        with tc.tile_pool(name="const", bufs=1) as cp:
            ident = cp.tile([128, 128], BF16)
            make_identity(nc, ident)
            identf = cp.tile([128, 128], FP32)
            make_identity(nc, identf)
            # lhsT tiles: block kc2 = k*2 + chalf holds rows c (128) x cout (256)
            wT = cp.tile([128, 2 * K, 256], BF16)
            # corner weights per token: (k, j, xpos, yrow, dup); bf16 dup
            # pairs give the blend's broadcast operand a stride-1 innermost
            # dim (enables the DVE 2x_1p perf mode)
            wpack = cp.tile([128, K, HW // 128, 2, 2, 2], BF16)
            # wrapped-16 gather indices per tap
            idx_w = cp.tile([128, K, HW // 16], I16)

            with (
                tc.tile_pool(name="prep", bufs=1) as pp,
                tc.tile_pool(name="ppsum", bufs=4, space="PSUM") as pps,
            ):
                # ---- x -> channels-last bf16 superpixel image in HBM ----
                x_bf = pp.tile([128, 2, HW], BF16)
                nc.gpsimd.dma_start(
                    out=x_bf[:], in_=x_in[:, :].rearrange("(h p) c -> p h c", p=128)
                )
                # zero borders first (rows -1/64 and cols -1/64 of the pad)
                zpad = pp.tile([128, 2 * C], BF16)
                nc.vector.memset(zpad[:], 0.0)
                # store-row 0, slot 0 (pixel row -1)
                nc.sync.dma_start(
                    out=AP(iz_d, 0, [[SP_ELEMS, NCOL], [1, C]]),
                    in_=zpad[0:NCOL, 0:C],
                )
                # store-row 64, slot 1 (pixel row 64)
                nc.sync.dma_start(
                    out=AP(iz_d, 64 * ROW_STRIDE + C, [[SP_ELEMS, NCOL], [1, C]]),
                    in_=zpad[0:NCOL, 0:C],
                )
                # store-col 0 (pixel col -1), both slots
                nc.sync.dma_start(
                    out=AP(iz_d, 0, [[ROW_STRIDE, NROW], [1, SP_ELEMS]]),
                    in_=zpad[0:NROW, :],
                )
                # store-col 65 (pixel col 64), both slots
                nc.sync.dma_start(
                    out=AP(iz_d, 65 * SP_ELEMS, [[ROW_STRIDE, NROW], [1, SP_ELEMS]]),
                    in_=zpad[0:NROW, :],
                )
                # ---- offsets -> indices + corner weights ----
                # (before the image transposes: the off->idx chain is the
                # critical path to the first gather; its 8 small PE transposes
                # go ahead of the 64 x-transposes, and the xcl PSUM->SBUF
                # copies run on ACT so DVE is free for this chain)
                # natural load (18 contiguous 16KB descriptors), then PE
                # transpose to token-major [128, j, r]
                off_nat = pp.tile([128, HW], FP32)
                nc.sync.dma_start(out=off_nat[0 : 2 * K, :], in_=off_in[:, :])
                off_t2 = pp.tile([128, HW // 128, 2 * K], FP32)
                for s4 in range(HW // 128 // 4):
                    ptf = pps.tile([128, 4, 2 * K], FP32, tag="ptf")
                    for i in range(4):
                        nc.tensor.transpose(
                            ptf[:, i, :],
                            off_nat[0 : 2 * K, ts(4 * s4 + i, 128)],
                            identf[0 : 2 * K, 0 : 2 * K],
                        )
                    nc.vector.tensor_copy(off_t2[:, 4 * s4 : 4 * s4 + 4, :], ptf[:])
                off_r = off_t2[:, :, :].rearrange("p j (k s) -> p s k j", s=2)
                oy = off_r[:, 0]  # [128, 9, 32] (k, j) strides (2, 18)
                ox = off_r[:, 1]

                NJ = HW // 128  # 32
                shp = [128, K, NJ]

                def f32(tag):
                    return pp.tile(shp, FP32, tag=tag, name=tag)

                # iotas
                it_j = pp.tile([128, NJ], I32)
                nc.gpsimd.iota(it_j[:], [[1, NJ]], base=0, channel_multiplier=0)
                jf = pp.tile([128, NJ], FP32)
                nc.vector.tensor_copy(jf[:], it_j[:])
                it_p = pp.tile([128, 1], I32)
                nc.gpsimd.iota(it_p[:], [[0, 1]], base=0, channel_multiplier=1)
                pf = pp.tile([128, 1], FP32)
                nc.vector.tensor_copy(pf[:], it_p[:])
                it_ky = pp.tile([128, 3, 3, NJ], I32)
                nc.gpsimd.iota(
                    it_ky[:], [[1, 3], [0, 3], [0, NJ]], base=0, channel_multiplier=0
                )
                kyf = pp.tile(shp, FP32, tag="kyf")
                nc.vector.tensor_copy(
                    kyf[:, :, :].rearrange("p (a b) j -> p a b j", a=3), it_ky[:]
                )
                it_kx = pp.tile([128, 3, 3, NJ], I32)
                nc.gpsimd.iota(
                    it_kx[:], [[0, 3], [1, 3], [0, NJ]], base=0, channel_multiplier=0
                )
                kxf = pp.tile(shp, FP32, tag="kxf")
                nc.vector.tensor_copy(
                    kxf[:, :, :].rearrange("p (a b) j -> p a b j", a=3), it_kx[:]
                )

                # ho = 2*j + p//64 ; wo = p%64  (token t = j*128 + p)
                t1 = pp.tile([128, 1], FP32, tag="t1")
                nc.vector.tensor_scalar(t1[:], pf[:], 1.0 / 64.0, None, A.mult)
                t2 = pp.tile([128, 1], FP32, tag="t2")
                nc.vector.tensor_scalar(t2[:], t1[:], 8388608.0, 8388608.0, A.add, A.subtract)
                p64 = pp.tile([128, 1], FP32, tag="p64")
                nc.vector.tensor_tensor(p64[:], t2[:], t1[:], A.is_gt)
                nc.vector.tensor_tensor(p64[:], t2[:], p64[:], A.subtract)
                wo = pp.tile([128, 1], FP32, tag="wo")
                nc.vector.tensor_scalar(wo[:], p64[:], -64.0, None, A.mult)
                nc.vector.tensor_tensor(wo[:], wo[:], pf[:], A.add)
                ho = pp.tile([128, NJ], FP32, tag="ho")
                nc.vector.tensor_scalar(ho[:], jf[:], 2.0, p64[:, 0:1], A.mult, A.add)

                # biased sample coords: pyb = oy + ky + ho + 7  (bias +8, base -1)
                pyb = f32("pyb")
                nc.vector.tensor_tensor(pyb[:], oy, kyf[:], A.add)
                nc.vector.scalar_tensor_tensor(
                    pyb[:], pyb[:], 7.0, ho[:].unsqueeze(1).broadcast_to(shp), A.add, A.add
                )
                pxb = f32("pxb")
                nc.vector.tensor_tensor(pxb[:], ox, kxf[:], A.add)
                nc.vector.scalar_tensor_tensor(
                    pxb[:], pxb[:], 7.0, wo[:].unsqueeze(1).broadcast_to(shp), A.add, A.add
                )

                fy = f32("fy")
                y0b = f32("y0b")
                nc.vector.tensor_scalar(fy[:], pyb[:], 8388608.0, 8388608.0, A.add, A.subtract)
                nc.vector.tensor_tensor(y0b[:], fy[:], pyb[:], A.is_gt)
                nc.vector.tensor_tensor(y0b[:], fy[:], y0b[:], A.subtract)
                nc.vector.tensor_tensor(fy[:], pyb[:], y0b[:], A.subtract)
                fx = f32("fx")
                x0b = f32("x0b")
                nc.vector.tensor_scalar(fx[:], pxb[:], 8388608.0, 8388608.0, A.add, A.subtract)
                nc.vector.tensor_tensor(x0b[:], fx[:], pxb[:], A.is_gt)
                nc.vector.tensor_tensor(x0b[:], fx[:], x0b[:], A.subtract)
                nc.vector.tensor_tensor(fx[:], pxb[:], x0b[:], A.subtract)

                ta = f32("ta")
                tb = f32("tb")
                # gather superpixel index:
                # idx = (clip(y0b,7,71)-7)*66 + clip(x0b,7,71)-7
                nc.vector.tensor_scalar(ta[:], y0b[:], 7.0, 71.0, A.max, A.min)
                nc.vector.tensor_scalar(tb[:], x0b[:], 7.0, 71.0, A.max, A.min)
                tc_ = f32("tc_")
                nc.vector.scalar_tensor_tensor(tc_[:], ta[:], 66.0, tb[:], A.mult, A.add)
                nc.vector.tensor_scalar(tc_[:], tc_[:], 469.0, None, A.subtract)
                idx16 = pp.tile([128, K, NJ], I16)
                nc.vector.tensor_copy(idx16[:], tc_[:])

                # token-major [p,(k,j)] -> p-major HBM staging
                nc.sync.dma_start(out=ilin_d[:, :], in_=idx16[:])
                # wrapped-16 load into group 0: idx for (k, c) lives at
                # p = (16c + p16) % 128, j = (16c + p16) // 128, decomposed
                # affine as c = 8*ch + cl
                nc.sync.dma_start(
                    out=idx_w[0:16, :, :].rearrange(
                        "p k (ch cl) -> p k ch cl", cl=8
                    ),
                    in_=AP(ilin_d, 0, [[288, 16], [32, K], [1, 32], [16 * 288, 8]]),
                )
                for g in range(1, 8):
                    nc.sync.dma_start(
                        out=idx_w[16 * g : 16 * (g + 1), :, :],
                        in_=idx_w[0:16, :, :],
                    )

                # validity-masked 1D weights:
                # wy0 = (1-fy)*[8<=y0b<=71], wy1 = fy*[7<=y0b<=70]
                nc.vector.tensor_scalar(ta[:], y0b[:], 8.0, None, A.is_ge)
                nc.vector.tensor_scalar(tb[:], y0b[:], 71.0, None, A.is_le)
                vy0 = f32("vy0")
                nc.vector.tensor_tensor(vy0[:], ta[:], tb[:], A.mult)
                nc.vector.tensor_scalar(ta[:], y0b[:], 7.0, None, A.is_ge)
                nc.vector.tensor_scalar(tb[:], y0b[:], 70.0, None, A.is_le)
                vy1 = f32("vy1")
                nc.vector.tensor_tensor(vy1[:], ta[:], tb[:], A.mult)
                u0 = f32("u0")
                nc.vector.tensor_scalar(u0[:], fy[:], -1.0, 1.0, A.mult, A.add)
                wy0 = f32("wy0")
                nc.vector.tensor_tensor(wy0[:], u0[:], vy0[:], A.mult)
                wy1 = f32("wy1")
                nc.vector.tensor_tensor(wy1[:], fy[:], vy1[:], A.mult)
                # wx0 = (1-fx)*[8<=x0b<=71], wx1 = fx*[7<=x0b<=70]
                nc.vector.tensor_scalar(ta[:], x0b[:], 8.0, None, A.is_ge)
                nc.vector.tensor_scalar(tb[:], x0b[:], 71.0, None, A.is_le)
                vx0 = f32("vx0")
                nc.vector.tensor_tensor(vx0[:], ta[:], tb[:], A.mult)
                nc.vector.tensor_scalar(ta[:], x0b[:], 7.0, None, A.is_ge)
                nc.vector.tensor_scalar(tb[:], x0b[:], 70.0, None, A.is_le)
                vx1 = f32("vx1")
                nc.vector.tensor_tensor(vx1[:], ta[:], tb[:], A.mult)
                nc.vector.tensor_scalar(u0[:], fx[:], -1.0, 1.0, A.mult, A.add)
                wx0 = f32("wx0")
                nc.vector.tensor_tensor(wx0[:], u0[:], vx0[:], A.mult)
                wx1 = f32("wx1")
                nc.vector.tensor_tensor(wx1[:], fx[:], vx1[:], A.mult)

                # corner weight products; corner order in the gathered
                # element: [A0 B0 A1 B1] = (x-slot outer, y-row inner)
                for dup in range(2):
                    nc.vector.tensor_tensor(
                        wpack[:, :, :, 0, 0, dup], wy0[:], wx0[:], A.mult
                    )
                    nc.vector.tensor_tensor(
                        wpack[:, :, :, 0, 1, dup], wy1[:], wx0[:], A.mult
                    )
                    nc.vector.tensor_tensor(
                        wpack[:, :, :, 1, 0, dup], wy0[:], wx1[:], A.mult
                    )
                    nc.vector.tensor_tensor(
                        wpack[:, :, :, 1, 1, dup], wy1[:], wx1[:], A.mult
                    )

                # ---- x -> channels-last superpixel image in HBM ----
                xcl_sb = pp.tile([128, HW // 128, C], BF16)
                # interior write AP dims: (ql 64, s_loc 8, c 256) per qh half,
                # matching the sbuf source [p=(qh ql), s_loc, c]; pixel
                # t = s*128+q, prow = 2s+qh (+16 per q4 group), pcol = ql.
                wdims = [
                    [2 * C, 64],
                    [2 * 132 * C, 8],
                    [1, C],
                ]
                for s in range(HW // 128):
                    pt = pps.tile([128, 2, 128], BF16, tag="pt")
                    for ch in range(2):
                        nc.tensor.transpose(
                            pt[:, ch, :], x_bf[:, ch, ts(s, 128)], ident[:]
                        )
                    nc.scalar.copy(xcl_sb[:, s, :], pt[:])
                    if s % 8 == 7:
                        q4 = s // 8
                        for qh in range(2):
                            src = xcl_sb[
                                64 * qh : 64 * (qh + 1), q4 * 8 : (q4 + 1) * 8, :
                            ]
                            base = (q4 * 16 + qh) * 132 * C
                            # split across SP (HWDGE) and Pool (SWDGE) queues
                            # so the writes don't serialize behind the idx
                            # staging chain on SP
                            eng = nc.sync if qh == 0 else nc.gpsimd
                            # slot 0: pixel row prow at store-row prow+1
                            eng.dma_start(
                                out=AP(iz_d, base + 134 * C, wdims), in_=src
                            )
                            # slot 1: pixel row prow at store-row prow
                            eng.dma_start(
                                out=AP(iz_d, base + 3 * C, wdims), in_=src
                            )

                # ---- weight lhsT (SWDGE load with fp32->bf16 cast) ----
                w_bf = pp.tile([128, 2, K * C], BF16)
                nc.gpsimd.dma_start(
                    out=w_bf[:], in_=w_in[:, :].rearrange("(h p) c -> p h c", p=128)
                )
                for kc2 in range(2 * K):
                    ptw = pps.tile([128, 2, 128], BF16, tag="pt")
                    for oh in range(2):
                        nc.tensor.transpose(
                            ptw[:, oh, :], w_bf[:, oh, ts(kc2, 128)], ident[:]
                        )
                    nc.scalar.copy(wT[:, kc2, :], ptw[:])


            # ---------------- main loop ----------------
            # gather source: start positions at superpixel granularity
            xsrc = AP(iz_d, 0, [[SP_ELEMS, NROW * NCOL - 1], [1, 2 * SP_ELEMS]])
            NJC = NCH // 128  # 8 j-columns per chunk
            nreg = nc.gpsimd.to_reg(NCH)

            with (
                tc.tile_pool(name="vp", bufs=5) as vp,
                tc.tile_pool(name="cc", bufs=6) as ccp,
                tc.tile_pool(name="ob", bufs=2) as obp,
                tc.tile_pool(name="mps", bufs=1, space="PSUM") as mps,
                tc.tile_pool(name="cps", bufs=2, space="PSUM") as cps,
            ):
                iters = [(nch, k) for nch in range(NCHUNKS) for k in range(K)]
                vts = {}

                def issue_gather(i):
                    nch_, k_ = iters[i]
                    v = vp.tile([128, NJC, 4 * C], BF16, tag="v", name="v")
                    if os.environ.get("KBISECT") == "1":
                        nc.vector.memset(v[:], 0.25)
                    else:
                        nc.gpsimd.dma_gather(
                            out_ap=v[:],
                            in_ap=xsrc,
                            idxs_ap=idx_w[
                                :,
                                k_,
                                nch_ * (NCH // 16) : (nch_ + 1) * (NCH // 16),
                            ],
                            num_idxs=NCH,
                            num_idxs_reg=nreg,
                            elem_size=2 * SP_ELEMS,
                            elem_step=SP_ELEMS,
                        )
                    vts[i] = v

                issue_gather(0)
                issue_gather(1)
                issue_gather(2)
                issue_gather(3)
                ps = None
                for it in range(len(iters)):
                    nch, k = iters[it]
                    if it + 4 < len(iters):
                        issue_gather(it + 4)
                    if k == 0:
                        ps = [
                            [mps.tile([128, 512], FP32, tag=f"ps{oh}{n2}", name=f"ps{oh}{n2}") for n2 in range(2)]
                            for oh in range(2)
                        ]
                    v = vts.pop(it)
                    # blend: v *= wpack (per-token corner weights) on DVE;
                    # the 4-corner sum then happens FOR FREE in PSUM via four
                    # accumulating transpose-matmuls per block
                    v6 = v[:, :, :].rearrange(
                        "p j (x y h two) -> p j x y h two", x=2, y=2, two=2
                    )
                    w6 = (
                        wpack[:, k, nch * NJC : (nch + 1) * NJC, :, :, :]
                        .unsqueeze(4)
                        .broadcast_to([128, NJC, 2, 2, C // 2, 2])
                    )
                    v4 = v[:, :, :].rearrange("p j (c4 c) -> p j c4 c", c4=4)
                    if os.environ.get("KBISECT") != "3":
                        nc.vector.tensor_tensor(v6, v6, w6, A.mult)

                    # weighted corners -> channel-major summed cols: PE
                    # matmuls against the identity transpose each corner
                    # block and accumulate the 4 corners in PSUM; one ACT
                    # PSUM->SBUF copy brings the cols back for the GEMM
                    cm = ccp.tile([128, 2, NCH], BF16, tag="cm")
                    if os.environ.get("KBISECT") in ("1", "2"):
                        nc.vector.memset(cm[:], 0.125)
                    else:
                        for ch in range(2):
                            cm_ps = cps.tile([128, NJC, 128], FP32, tag="cmps")
                            for j in range(NJC):
                                for c4 in range(4):
                                    nc.tensor.matmul(
                                        cm_ps[:, j, :],
                                        v4[:, j, c4, ts(ch, 128)],
                                        ident[:],
                                        start=(c4 == 0),
                                        stop=(c4 == 3),
                                    )
                            nc.scalar.copy(cm[:, ch, :], cm_ps[:])

                    for oh in range(2):
                        for ch in range(2):
                            lhsT = wT[:, 2 * k + ch, ts(oh, 128)]
                            for n2 in range(2):
                                nc.tensor.matmul(
                                    ps[oh][n2][:],
                                    lhsT,
                                    cm[:, ch, ts(n2, 512)],
                                    start=(k == 0 and ch == 0),
                                    stop=(k == K - 1 and ch == 1),
                                )

                    if k != K - 1:
                        continue
                    ob = obp.tile([128, 2, NCH], FP32, tag="ob")
                    for oh in range(2):
                        for n2 in range(2):
                            nc.scalar.copy(ob[:, oh, ts(n2, 512)], ps[oh][n2][:])
                    for oh in range(2):
                        for n2 in range(2):
                            nc.sync.dma_start(
                                out=out_d[
                                    128 * oh : 128 * (oh + 1),
                                    nch * NCH + 512 * n2 : nch * NCH + 512 * (n2 + 1),
                                ],
                                in_=ob[:, oh, ts(n2, 512)],
                            )
    nc.compile()
    return nc


_NC_CACHE = None


def _get_nc():
    global _NC_CACHE
    if _NC_CACHE is None:
        _NC_CACHE = build_nc()
    return _NC_CACHE


def kernel(x: np.ndarray, offset: np.ndarray, weight: np.ndarray) -> np.ndarray:
    return _run(x, offset, weight)[0]


def _run(x, offset, weight, **spmd_kwargs):
    assert x.shape == (B, C, H, W) and offset.shape == (B, 2 * K, H, W)
    nc = _get_nc()
    # [cout, cin, 3, 3] -> [cout, (k, c)]
    w_perm = np.ascontiguousarray(
        weight.reshape(COUT, C, K).transpose(0, 2, 1).reshape(COUT, K * C)
    ).astype(np.float32)
    in_maps = [
        {
            "x": np.ascontiguousarray(x[b].reshape(C, HW)).astype(np.float32),
            "off": np.ascontiguousarray(offset[b].reshape(2 * K, HW)).astype(
                np.float32
            ),
            "w": w_perm,
        }
        for b in range(B)
    ]
    res = run_bass_kernel_spmd(nc, in_maps, core_ids=list(range(B)), **spmd_kwargs)
    out = np.stack([res.results[b]["out"].reshape(COUT, H, W) for b in range(B)])
    return out.astype(np.float32), res


if __name__ == "__main__":
    d = np.load("/root/problem/inputs.npz")
    out = kernel(d["x"], d["offset"], d["weight"])
    ref = np.load("/root/problem/ref_out_np.npy")
    err = np.abs(out - ref).max()
    rel = err / np.abs(ref).max()
    print("absmax err:", err, "rel:", rel)


# revision 59
# speedup vs baseline: 1.0943x; 1.0070x over previous
"""Deformable conv (3x3, pad=1, B=8, Cin=Cout=256, H=W=64) on 8 TRN2 NeuronCores.

Strategy (data-parallel over batch, one image per core):
  1. Prologue per core: build a zero-padded *superpixel* image IZ in HBM via PE
     transpose: IZ[R, X] = [pix(R-1, X-1), pix(R, X-1)] (vertical pixel pair,
     512 ch contiguous, bf16), R in 0..64, X in 0..65, with zero rows/cols on
     the borders.  One 2KB gather descriptor starting at superpixel (R, X)
     then fetches all 4 bilinear corners [A0 B0 A1 B1] of a tap sample.
     Also: transpose the (k,c)-ordered weight matrix into lhsT layout; compute
     per-token superpixel indices + 4 corner weights on DVE from the offsets.
  2. Main loop over (token-chunk, tap): ONE SWDGE dma_gather fetches the 4
     corners per output token (token-major: token on partition, 4x256 channels
     on free dim).  DVE multiplies the corners by per-token weights (free-dim
     step-0 broadcast APs with dup-pairs for the 2x_1p perf mode) and does the
     first pair-sum; GPSIMD does the second pair-sum.  PE block-transposes the
     blended tile to channel-major [c, token] in PSUM (ACT copies it back to
     SBUF), keeping the SDMA engines free for the main gather.  TensorE
     accumulates W_k^T @ cols into PSUM over all 9 taps x 2 c-halves.
  3. PSUM -> SBUF -> HBM fp32 output.

Boundary handling is entirely via the zero padding: the gather index is
(clip(y0,-1,63)+1)*66 + clip(x0,-1,63)+1 and out-of-image corners get weight 0
through the validity masks (exactly the reference's `valid` mask).
"""

import os

import numpy as np

import concourse.bacc as bacc
import concourse.bass as bass
import concourse.mybir as mybir
from concourse.bass import AP, ts
from concourse.bass_utils import run_bass_kernel_spmd
from concourse.masks import make_identity
from concourse.tile import TileContext

FP32 = mybir.dt.float32
BF16 = mybir.dt.bfloat16
I16 = mybir.dt.int16
I32 = mybir.dt.int32

B = 8
C = 256
H = W = 64
HW = H * W           # 4096 pixels / tokens per tap
K = 9                # 3x3 taps
COUT = 256
NCH = 1024           # tokens per chunk
NCHUNKS = HW // NCH  # 4
A = mybir.AluOpType

# superpixel table geometry: 65 store-rows (R=0..64), 66 store-cols (X=0..65),
# 2 pixel slots of 256 ch each.  flat [65*66*2, 256] bf16.
NROW = 65
NCOL = 66
SP_ELEMS = 2 * C                 # 512 elements per superpixel
IZ_ROWS = NROW * NCOL * 2        # 8580 rows of 256
ROW_STRIDE = NCOL * SP_ELEMS     # 33792 elems per store-row


def build_nc() -> bass.Bass:
    nc = bacc.Bacc(target_bir_lowering=False)

    x_in = nc.dram_tensor("x", [C, HW], FP32, kind="ExternalInput")
    off_in = nc.dram_tensor("off", [2 * K, HW], FP32, kind="ExternalInput")
    # host pre-permutes weight to [cout, (k, c)] layout
    w_in = nc.dram_tensor("w", [COUT, K * C], FP32, kind="ExternalInput")
    out_d = nc.dram_tensor("out", [COUT, HW], FP32, kind="ExternalOutput")

    # zero-padded superpixel image (see module docstring)
    iz_d = nc.dram_tensor("iz", [IZ_ROWS, C], BF16, kind="Internal")
    # token-major -> wrapped-16 index staging
    ilin_d = nc.dram_tensor("ilin", [128, K * HW // 128], I16, kind="Internal")

    with TileContext(nc) as tc:
        with tc.tile_pool(name="const", bufs=1) as cp:
            ident = cp.tile([128, 128], BF16)
            make_identity(nc, ident)
            identf = cp.tile([128, 128], FP32)
            make_identity(nc, identf)
            # lhsT tiles: block kc2 = k*2 + chalf holds rows c (128) x cout (256)
            wT = cp.tile([128, 2 * K, 256], BF16)
            # corner weights per token: (k, j, xpos, yrow, dup); bf16 dup
            # pairs give the blend's broadcast operand a stride-1 innermost
            # dim (enables the DVE 2x_1p perf mode)
            wpack = cp.tile([128, K, HW // 128, 2, 2, 2], BF16)
            # wrapped-16 gather indices per tap
            idx_w = cp.tile([128, K, HW // 16], I16)

            with (
                tc.tile_pool(name="prep", bufs=1) as pp,
                tc.tile_pool(name="ppsum", bufs=4, space="PSUM") as pps,
            ):
                # ---- x -> channels-last bf16 superpixel image in HBM ----
                # (two half-loads so the first transposes start sooner)
                x_bf = pp.tile([128, 2, HW], BF16)
                for xh in range(2):
                    nc.gpsimd.dma_start(
                        out=x_bf[:, :, xh * (HW // 2) : (xh + 1) * (HW // 2)],
                        in_=x_in[:, xh * (HW // 2) : (xh + 1) * (HW // 2)].rearrange(
                            "(h p) c -> p h c", p=128
                        ),
                    )
                # zero borders first (rows -1/64 and cols -1/64 of the pad)
                zpad = pp.tile([128, 2 * C], BF16)
                nc.vector.memset(zpad[:], 0.0)
                # store-row 0, slot 0 (pixel row -1)
                nc.sync.dma_start(
                    out=AP(iz_d, 0, [[SP_ELEMS, NCOL], [1, C]]),
                    in_=zpad[0:NCOL, 0:C],
                )
                # store-row 64, slot 1 (pixel row 64)
                nc.sync.dma_start(
                    out=AP(iz_d, 64 * ROW_STRIDE + C, [[SP_ELEMS, NCOL], [1, C]]),
                    in_=zpad[0:NCOL, 0:C],
                )
                # store-col 0 (pixel col -1), both slots
                nc.sync.dma_start(
                    out=AP(iz_d, 0, [[ROW_STRIDE, NROW], [1, SP_ELEMS]]),
                    in_=zpad[0:NROW, :],
                )
                # store-col 65 (pixel col 64), both slots
                nc.sync.dma_start(
                    out=AP(iz_d, 65 * SP_ELEMS, [[ROW_STRIDE, NROW], [1, SP_ELEMS]]),
                    in_=zpad[0:NROW, :],
                )
                # ---- offsets -> indices + corner weights ----
                # (before the image transposes: the off->idx chain is the
                # critical path to the first gather; its 8 small PE transposes
                # go ahead of the 64 x-transposes, and the xcl PSUM->SBUF
                # copies run on ACT so DVE is free for this chain)
                # natural load (18 contiguous 16KB descriptors), then PE
                # transpose to token-major [128, j, r]
                off_nat = pp.tile([128, HW], FP32)
                nc.sync.dma_start(out=off_nat[0 : 2 * K, :], in_=off_in[:, :])
                off_t2 = pp.tile([128, HW // 128, 2 * K], FP32)
                for s4 in range(HW // 128 // 4):
                    ptf = pps.tile([128, 4, 2 * K], FP32, tag="ptf")
                    for i in range(4):
                        nc.tensor.transpose(
                            ptf[:, i, :],
                            off_nat[0 : 2 * K, ts(4 * s4 + i, 128)],
                            identf[0 : 2 * K, 0 : 2 * K],
                        )
                    nc.vector.tensor_copy(off_t2[:, 4 * s4 : 4 * s4 + 4, :], ptf[:])
                off_r = off_t2[:, :, :].rearrange("p j (k s) -> p s k j", s=2)
                oy = off_r[:, 0]  # [128, 9, 32] (k, j) strides (2, 18)
                ox = off_r[:, 1]

                NJ = HW // 128  # 32
                shp = [128, K, NJ]

                def f32(tag):
                    return pp.tile(shp, FP32, tag=tag, name=tag)

                # iotas
                it_j = pp.tile([128, NJ], I32)
                nc.gpsimd.iota(it_j[:], [[1, NJ]], base=0, channel_multiplier=0)
                jf = pp.tile([128, NJ], FP32)
                nc.vector.tensor_copy(jf[:], it_j[:])
                it_p = pp.tile([128, 1], I32)
                nc.gpsimd.iota(it_p[:], [[0, 1]], base=0, channel_multiplier=1)
                pf = pp.tile([128, 1], FP32)
                nc.vector.tensor_copy(pf[:], it_p[:])
                it_ky = pp.tile([128, 3, 3, NJ], I32)
                nc.gpsimd.iota(
                    it_ky[:], [[1, 3], [0, 3], [0, NJ]], base=0, channel_multiplier=0
                )
                kyf = pp.tile(shp, FP32, tag="kyf")
                nc.vector.tensor_copy(
                    kyf[:, :, :].rearrange("p (a b) j -> p a b j", a=3), it_ky[:]
                )
                it_kx = pp.tile([128, 3, 3, NJ], I32)
                nc.gpsimd.iota(
                    it_kx[:], [[0, 3], [1, 3], [0, NJ]], base=0, channel_multiplier=0
                )
                kxf = pp.tile(shp, FP32, tag="kxf")
                nc.vector.tensor_copy(
                    kxf[:, :, :].rearrange("p (a b) j -> p a b j", a=3), it_kx[:]
                )

                # ho = 2*j + p//64 ; wo = p%64  (token t = j*128 + p)
                t1 = pp.tile([128, 1], FP32, tag="t1")
                nc.vector.tensor_scalar(t1[:], pf[:], 1.0 / 64.0, None, A.mult)
                t2 = pp.tile([128, 1], FP32, tag="t2")
                nc.vector.tensor_scalar(t2[:], t1[:], 8388608.0, 8388608.0, A.add, A.subtract)
                p64 = pp.tile([128, 1], FP32, tag="p64")
                nc.vector.tensor_tensor(p64[:], t2[:], t1[:], A.is_gt)
                nc.vector.tensor_tensor(p64[:], t2[:], p64[:], A.subtract)
                wo = pp.tile([128, 1], FP32, tag="wo")
                nc.vector.tensor_scalar(wo[:], p64[:], -64.0, None, A.mult)
                nc.vector.tensor_tensor(wo[:], wo[:], pf[:], A.add)
                ho = pp.tile([128, NJ], FP32, tag="ho")
                nc.vector.tensor_scalar(ho[:], jf[:], 2.0, p64[:, 0:1], A.mult, A.add)

                # biased sample coords: pyb = oy + ky + ho + 7  (bias +8, base -1)
                pyb = f32("pyb")
                nc.vector.tensor_tensor(pyb[:], oy, kyf[:], A.add)
                nc.vector.scalar_tensor_tensor(
                    pyb[:], pyb[:], 7.0, ho[:].unsqueeze(1).broadcast_to(shp), A.add, A.add
                )
                pxb = f32("pxb")
                nc.vector.tensor_tensor(pxb[:], ox, kxf[:], A.add)
                nc.vector.scalar_tensor_tensor(
                    pxb[:], pxb[:], 7.0, wo[:].unsqueeze(1).broadcast_to(shp), A.add, A.add
                )

                fy = f32("fy")
                y0b = f32("y0b")
                nc.vector.tensor_scalar(fy[:], pyb[:], 8388608.0, 8388608.0, A.add, A.subtract)
                nc.vector.tensor_tensor(y0b[:], fy[:], pyb[:], A.is_gt)
                nc.vector.tensor_tensor(y0b[:], fy[:], y0b[:], A.subtract)
                nc.vector.tensor_tensor(fy[:], pyb[:], y0b[:], A.subtract)
                fx = f32("fx")
                x0b = f32("x0b")
                nc.vector.tensor_scalar(fx[:], pxb[:], 8388608.0, 8388608.0, A.add, A.subtract)
                nc.vector.tensor_tensor(x0b[:], fx[:], pxb[:], A.is_gt)
                nc.vector.tensor_tensor(x0b[:], fx[:], x0b[:], A.subtract)
                nc.vector.tensor_tensor(fx[:], pxb[:], x0b[:], A.subtract)

                ta = f32("ta")
                tb = f32("tb")
                # gather superpixel index:
                # idx = (clip(y0b,7,71)-7)*66 + clip(x0b,7,71)-7
                nc.vector.tensor_scalar(ta[:], y0b[:], 7.0, 71.0, A.max, A.min)
                nc.vector.tensor_scalar(tb[:], x0b[:], 7.0, 71.0, A.max, A.min)
                tc_ = f32("tc_")
                nc.vector.scalar_tensor_tensor(tc_[:], ta[:], 66.0, tb[:], A.mult, A.add)
                nc.vector.tensor_scalar(tc_[:], tc_[:], 469.0, None, A.subtract)
                idx16 = pp.tile([128, K, NJ], I16)
                nc.vector.tensor_copy(idx16[:], tc_[:])

                # token-major [p,(k,j)] -> p-major HBM staging
                nc.sync.dma_start(out=ilin_d[:, :], in_=idx16[:])
                # wrapped-16 load into group 0: idx for (k, c) lives at
                # p = (16c + p16) % 128, j = (16c + p16) // 128, decomposed
                # affine as c = 8*ch + cl
                nc.sync.dma_start(
                    out=idx_w[0:16, :, :].rearrange(
                        "p k (ch cl) -> p k ch cl", cl=8
                    ),
                    in_=AP(ilin_d, 0, [[288, 16], [32, K], [1, 32], [16 * 288, 8]]),
                )
                for g in range(1, 8):
                    nc.sync.dma_start(
                        out=idx_w[16 * g : 16 * (g + 1), :, :],
                        in_=idx_w[0:16, :, :],
                    )

                # ---- x -> channels-last superpixel image in HBM ----
                xcl_sb = pp.tile([128, HW // 128, C], BF16)
                # interior write AP dims: (ql 64, s_loc 8, c 256) per qh half,
                # matching the sbuf source [p=(qh ql), s_loc, c]; pixel
                # t = s*128+q, prow = 2s+qh (+16 per q4 group), pcol = ql.
                wdims = [
                    [2 * C, 64],
                    [2 * 132 * C, 8],
                    [1, C],
                ]
                for s in range(HW // 128):
                    pt = pps.tile([128, 2, 128], BF16, tag="pt")
                    for ch in range(2):
                        nc.tensor.transpose(
                            pt[:, ch, :], x_bf[:, ch, ts(s, 128)], ident[:]
                        )
                    if s < 16:
                        nc.scalar.copy(xcl_sb[:, s, :], pt[:])
                    else:
                        nc.vector.tensor_copy(xcl_sb[:, s, :], pt[:])
                    if s % 8 == 7:
                        q4 = s // 8
                        for qh in range(2):
                            src = xcl_sb[
                                64 * qh : 64 * (qh + 1), q4 * 8 : (q4 + 1) * 8, :
                            ]
                            base = (q4 * 16 + qh) * 132 * C
                            # split across SP (HWDGE) and Pool (SWDGE) queues
                            # so the writes don't serialize behind the idx
                            # staging chain on SP
                            eng = nc.sync if qh == 0 else nc.gpsimd
                            # slot 0: pixel row prow at store-row prow+1
                            eng.dma_start(
                                out=AP(iz_d, base + 134 * C, wdims), in_=src
                            )
                            # slot 1: pixel row prow at store-row prow
                            eng.dma_start(
                                out=AP(iz_d, base + 3 * C, wdims), in_=src
                            )

                # validity-masked 1D weights:
                # wy0 = (1-fy)*[8<=y0b<=71], wy1 = fy*[7<=y0b<=70]
                nc.vector.tensor_scalar(ta[:], y0b[:], 8.0, None, A.is_ge)
                nc.vector.tensor_scalar(tb[:], y0b[:], 71.0, None, A.is_le)
                vy0 = f32("vy0")
                nc.vector.tensor_tensor(vy0[:], ta[:], tb[:], A.mult)
                nc.vector.tensor_scalar(ta[:], y0b[:], 7.0, None, A.is_ge)
                nc.vector.tensor_scalar(tb[:], y0b[:], 70.0, None, A.is_le)
                vy1 = f32("vy1")
                nc.vector.tensor_tensor(vy1[:], ta[:], tb[:], A.mult)
                u0 = f32("u0")
                nc.vector.tensor_scalar(u0[:], fy[:], -1.0, 1.0, A.mult, A.add)
                wy0 = f32("wy0")
                nc.vector.tensor_tensor(wy0[:], u0[:], vy0[:], A.mult)
                wy1 = f32("wy1")
                nc.vector.tensor_tensor(wy1[:], fy[:], vy1[:], A.mult)
                # wx0 = (1-fx)*[8<=x0b<=71], wx1 = fx*[7<=x0b<=70]
                nc.vector.tensor_scalar(ta[:], x0b[:], 8.0, None, A.is_ge)
                nc.vector.tensor_scalar(tb[:], x0b[:], 71.0, None, A.is_le)
                vx0 = f32("vx0")
                nc.vector.tensor_tensor(vx0[:], ta[:], tb[:], A.mult)
                nc.vector.tensor_scalar(ta[:], x0b[:], 7.0, None, A.is_ge)
                nc.vector.tensor_scalar(tb[:], x0b[:], 70.0, None, A.is_le)
                vx1 = f32("vx1")
                nc.vector.tensor_tensor(vx1[:], ta[:], tb[:], A.mult)
                nc.vector.tensor_scalar(u0[:], fx[:], -1.0, 1.0, A.mult, A.add)
                wx0 = f32("wx0")
                nc.vector.tensor_tensor(wx0[:], u0[:], vx0[:], A.mult)
                wx1 = f32("wx1")
                nc.vector.tensor_tensor(wx1[:], fx[:], vx1[:], A.mult)

                # corner weight products; corner order in the gathered
                # element: [A0 B0 A1 B1] = (x-slot outer, y-row inner)
                for dup in range(2):
                    nc.vector.tensor_tensor(
                        wpack[:, :, :, 0, 0, dup], wy0[:], wx0[:], A.mult
                    )
                    nc.vector.tensor_tensor(
                        wpack[:, :, :, 0, 1, dup], wy1[:], wx0[:], A.mult
                    )
                    nc.vector.tensor_tensor(
                        wpack[:, :, :, 1, 0, dup], wy0[:], wx1[:], A.mult
                    )
                    nc.vector.tensor_tensor(
                        wpack[:, :, :, 1, 1, dup], wy1[:], wx1[:], A.mult
                    )

                # ---- weight lhsT (SWDGE load with fp32->bf16 cast) ----
                w_bf = pp.tile([128, 2, K * C], BF16)
                nc.gpsimd.dma_start(
                    out=w_bf[:], in_=w_in[:, :].rearrange("(h p) c -> p h c", p=128)
                )
                for kc2 in range(2 * K):
                    ptw = pps.tile([128, 2, 128], BF16, tag="pt")
                    for oh in range(2):
                        nc.tensor.transpose(
                            ptw[:, oh, :], w_bf[:, oh, ts(kc2, 128)], ident[:]
                        )
                    nc.scalar.copy(wT[:, kc2, :], ptw[:])


            # ---------------- main loop ----------------
            # gather source: start positions at superpixel granularity
            xsrc = AP(iz_d, 0, [[SP_ELEMS, NROW * NCOL - 1], [1, 2 * SP_ELEMS]])
            NJC = NCH // 128  # 8 j-columns per chunk
            nreg = nc.gpsimd.to_reg(NCH)

            with (
                tc.tile_pool(name="vp", bufs=5) as vp,
                tc.tile_pool(name="cc", bufs=6) as ccp,
                tc.tile_pool(name="ob", bufs=2) as obp,
                tc.tile_pool(name="mps", bufs=1, space="PSUM") as mps,
                tc.tile_pool(name="cps", bufs=2, space="PSUM") as cps,
            ):
                iters = [(nch, k) for nch in range(NCHUNKS) for k in range(K)]
                vts = {}

                def issue_gather(i):
                    nch_, k_ = iters[i]
                    v = vp.tile([128, NJC, 4 * C], BF16, tag="v", name="v")
                    if os.environ.get("KBISECT") == "1":
                        nc.vector.memset(v[:], 0.25)
                    else:
                        nc.gpsimd.dma_gather(
                            out_ap=v[:],
                            in_ap=xsrc,
                            idxs_ap=idx_w[
                                :,
                                k_,
                                nch_ * (NCH // 16) : (nch_ + 1) * (NCH // 16),
                            ],
                            num_idxs=NCH,
                            num_idxs_reg=nreg,
                            elem_size=2 * SP_ELEMS,
                            elem_step=SP_ELEMS,
                        )
                    vts[i] = v

                issue_gather(0)
                issue_gather(1)
                issue_gather(2)
                issue_gather(3)
                ps = None
                for it in range(len(iters)):
                    nch, k = iters[it]
                    if it + 4 < len(iters):
                        issue_gather(it + 4)
                    if k == 0:
                        ps = [
                            [mps.tile([128, 512], FP32, tag=f"ps{oh}{n2}", name=f"ps{oh}{n2}") for n2 in range(2)]
                            for oh in range(2)
                        ]
                    v = vts.pop(it)
                    # blend: v *= wpack (per-token corner weights) on DVE;
                    # the 4-corner sum then happens FOR FREE in PSUM via four
                    # accumulating transpose-matmuls per block
                    v6 = v[:, :, :].rearrange(
                        "p j (x y h two) -> p j x y h two", x=2, y=2, two=2
                    )
                    w6 = (
                        wpack[:, k, nch * NJC : (nch + 1) * NJC, :, :, :]
                        .unsqueeze(4)
                        .broadcast_to([128, NJC, 2, 2, C // 2, 2])
                    )
                    v4 = v[:, :, :].rearrange("p j (c4 c) -> p j c4 c", c4=4)
                    if os.environ.get("KBISECT") != "3":
                        nc.vector.tensor_tensor(v6, v6, w6, A.mult)

                    # weighted corners -> channel-major summed cols: PE
                    # matmuls against the identity transpose each corner
                    # block and accumulate the 4 corners in PSUM; one ACT
                    # PSUM->SBUF copy brings the cols back for the GEMM
                    cm = ccp.tile([128, 2, NCH], BF16, tag="cm")
                    if os.environ.get("KBISECT") in ("1", "2"):
                        nc.vector.memset(cm[:], 0.125)
                    else:
                        for ch in range(2):
                            cm_ps = cps.tile([128, NJC, 128], FP32, tag="cmps")
                            for j in range(NJC):
                                for c4 in range(4):
                                    nc.tensor.matmul(
                                        cm_ps[:, j, :],
                                        v4[:, j, c4, ts(ch, 128)],
                                        ident[:],
                                        start=(c4 == 0),
                                        stop=(c4 == 3),
                                    )
                            nc.scalar.copy(cm[:, ch, :], cm_ps[:])

                    for oh in range(2):
                        for ch in range(2):
                            lhsT = wT[:, 2 * k + ch, ts(oh, 128)]
                            for n2 in range(2):
                                nc.tensor.matmul(
                                    ps[oh][n2][:],
                                    lhsT,
                                    cm[:, ch, ts(n2, 512)],
                                    start=(k == 0 and ch == 0),
                                    stop=(k == K - 1 and ch == 1),
                                )

                    if k != K - 1:
                        continue
                    ob = obp.tile([128, 2, NCH], FP32, tag="ob")
                    for oh in range(2):
                        for n2 in range(2):
                            nc.scalar.copy(ob[:, oh, ts(n2, 512)], ps[oh][n2][:])
                    for oh in range(2):
                        for n2 in range(2):
                            nc.sync.dma_start(
                                out=out_d[
                                    128 * oh : 128 * (oh + 1),
                                    nch * NCH + 512 * n2 : nch * NCH + 512 * (n2 + 1),
                                ],
                                in_=ob[:, oh, ts(n2, 512)],
                            )
    nc.compile()
    return nc


_NC_CACHE = None


def _get_nc():
    global _NC_CACHE
    if _NC_CACHE is None:
        _NC_CACHE = build_nc()
    return _NC_CACHE


def kernel(x: np.ndarray, offset: np.ndarray, weight: np.ndarray) -> np.ndarray:
    return _run(x, offset, weight)[0]


def _run(x, offset, weight, **spmd_kwargs):
    assert x.shape == (B, C, H, W) and offset.shape == (B, 2 * K, H, W)
    nc = _get_nc()
    # [cout, cin, 3, 3] -> [cout, (k, c)]
    w_perm = np.ascontiguousarray(
        weight.reshape(COUT, C, K).transpose(0, 2, 1).reshape(COUT, K * C)
    ).astype(np.float32)
    in_maps = [
        {
            "x": np.ascontiguousarray(x[b].reshape(C, HW)).astype(np.float32),
            "off": np.ascontiguousarray(offset[b].reshape(2 * K, HW)).astype(
                np.float32
            ),
            "w": w_perm,
        }
        for b in range(B)
    ]
    res = run_bass_kernel_spmd(nc, in_maps, core_ids=list(range(B)), **spmd_kwargs)
    out = np.stack([res.results[b]["out"].reshape(COUT, H, W) for b in range(B)])
    return out.astype(np.float32), res


if __name__ == "__main__":
    d = np.load("/root/problem/inputs.npz")
    out = kernel(d["x"], d["offset"], d["weight"])
    ref = np.load("/root/problem/ref_out_np.npy")
    err = np.abs(out - ref).max()
    rel = err / np.abs(ref).max()
    print("absmax err:", err, "rel:", rel)
